# revision 19
# baseline (speedup 1.0000x reference)
"""Trainium2 Bass kernel for the separable transpose-conv (wavelet synthesis) layer.

Full op: x [16, 128, 128, 144] f32 -> out [16, 256, 256, 16] f32.
Two passes of grouped 1D transpose convs (stride 2, 9 taps, 3ch->1ch) with
symmetric padding + border multipliers, separable over W then H.

Formulation: each pass folds (symmetric pad + border multiplier + polyphase
transpose conv + crop) into a constant banded matrix A[cc] of shape [128, 256]
per within-triplet channel cc (columns 0:128 = even outputs, 128:256 = odd).

  pass 1 (W):  z[b,h,g,v]   = sum_w sum_cc x[b,h,w,3g+cc] * A[cc][w,v]
  pass 2 (H):  o[b,m,G2,v]  = sum_h sum_gg A[gg][h,m] * z[b,h,3G2+gg,v]

Both passes map onto PE matmuls with the spatial conv axis as the contraction
(partition) dim; the 3-way channel mixing becomes 3 PSUM-accumulated matmuls.
H == W == 128 so the same A matrices serve both passes.

Perf notes (from NTFF traces):
- Tensor engine is the critical path: 288 N=256 matmuls (pass 1, 109ns each)
  + 96 N=512 matmuls (pass 2, 215ns each) ~= 52us streaming floor per core.
  LDWEIGHTS are hidden behind streaming; banded/K-packed reformulations lose
  to the LDWEIGHTS rate or need partition-crossing transposes, so dense wins.
- The PE runs at ~50% duty for the first ~11us of tensor activity (power-ramp
  throttle: throttle_activity_1 util_limit=0.5). Dummy warm-up matmuls during
  the otherwise-idle head (framework preamble + first x-chunk DMA) trigger the
  ramp early so real matmuls run at full rate.
- Output is stored as bf16 and upcast on the host (tolerance is 2e-2; bf16
  rounding is ~4e-3), halving store traffic and the end-of-kernel drain.

Sharding: pure data parallel, batch 16 -> 2 per core across 8 cores (SPMD).
"""

import os
import numpy as np

N_CORES = 8
B_FULL = 16
B_PER = B_FULL // N_CORES  # 2
H = 128
W = 128
C = 144
G = C // 3    # 48
G2 = C // 9   # 16

# Number of dummy PE warm-up matmuls (N=512 each, ~215ns) issued before the
# first real matmul to start the power-management ramp during the DMA head.
N_WARM = int(os.environ.get("KWARM", "9"))
C0 = 12  # channels of batch-0 x fused into the amat load (one DMA launch)


def _build_A():
    """A [3, 128, 256] f32: banded matrices with pad reflection + border
    multiplier folded in. Validated against the jax reference to ~1e-7 rel."""
    t = np.arange(27, dtype=np.float64).reshape(3, 9)
    inv = (np.cos(t * np.float32(0.7)).astype(np.float32) * 0.5).astype(np.float32)

    L = 128
    P = L + 6
    R = np.zeros((P, L), np.float32)
    R[0, 2] = 2.0
    R[1, 1] = 1.5
    R[2, 0] = 1.25
    for i in range(L):
        R[3 + i, i] = 1.0
    R[P - 3, L - 1] = 1.25
    R[P - 2, L - 2] = 1.5
    R[P - 1, L - 3] = 2.0

    A = np.zeros((3, L, 256), np.float32)
    for cc in range(3):
        Me = np.zeros((P, L), np.float32)
        Mo = np.zeros((P, L), np.float32)
        for v in range(L):
            for j in range(5):
                Me[v + 5 - j, v] += inv[cc, 2 * j]
            for j in range(4):
                Mo[v + 5 - j, v] += inv[cc, 2 * j + 1]
        A[cc, :, 0:128] = R.T @ Me
        A[cc, :, 128:256] = R.T @ Mo
    return A


_CACHE = {}


def _get_nc():
    if "nc" in _CACHE:
        return _CACHE["nc"]

    import concourse.bacc as bacc
    import concourse.tile as tile
    from concourse import mybir

    f32 = mybir.dt.float32
    dt_mm = mybir.dt.bfloat16

    nc = bacc.Bacc("TRN2", target_bir_lowering=False, debug=False, num_devices=N_CORES)
    # x arrives host-pre-transposed to [b, w, c, h] so every DMA descriptor is a
    # full contiguous per-partition run and pass-1 weight slices are contiguous.
    x_ext = nc.declare_dram_parameter("x", [B_PER, W, C, H], dt_mm, isOutput=False)
    # amat (768 cols) fused with batch-0 channels [0:C0) of x (C0*128 cols):
    # one DMA launch covers everything the first matmuls need.
    a_ext = nc.declare_dram_parameter("amat", [128, 3 * 256 + C0 * H], dt_mm, isOutput=False)
    o_ext = nc.declare_dram_parameter("out", [B_PER, 2 * H, 2 * W, G2], dt_mm, isOutput=True)

    with tile.TileContext(nc) as tc:
        with tc.tile_pool(name="const", bufs=1) as cpool, \
             tc.tile_pool(name="xp", bufs=2) as xpool, \
             tc.tile_pool(name="yp", bufs=1) as ypool, \
             tc.tile_pool(name="st", bufs=2) as spool, \
             tc.tile_pool(name="zp", bufs=4, space="PSUM") as zpool, \
             tc.tile_pool(name="op", bufs=3, space="PSUM") as opool, \
             tc.tile_pool(name="wp", bufs=1, space="PSUM") as wpool:

            # ---- PE warm-up: trigger the power ramp before real work ----
            warm = cpool.tile([128, 512], dt_mm, tag="warm")
            nc.gpsimd.memset(warm[:], 1.0)
            wp = wpool.tile([128, 512], f32, tag="wp")
            for _ in range(N_WARM):
                nc.tensor.matmul(
                    out=wp[:], lhsT=warm[:, 0:128], rhs=warm[:],
                    start=True, stop=True,
                )

            # amat (+ fused first x block) first on the fast sync HWDGE ring:
            # it gates every matmul.
            amat = cpool.tile([128, 3 * 256 + C0 * H], dt_mm, tag="amat")
            nc.sync.dma_start(out=amat[:], in_=a_ext[:])
            amat_mm = amat[:]

            for b in range(B_PER):
                # ---- load x[b]: SBUF [w; (c, h)], contiguous; c-blocks pipeline pass 1 ----
                x_sb = xpool.tile([128, C, H], dt_mm, tag="x")
                # batch 0: channels [0:C0) already arrive fused with amat;
                # fine-grained chunks keep pass-1 fed just-in-time
                bounds = ([C0, 24, 36, 48, 72, 96, 120, 144] if b == 0
                          else [0, 48, 96, 144])
                for c0, c1 in zip(bounds, bounds[1:]):
                    nc.sync.dma_start(
                        out=x_sb[:, c0:c1, :],
                        in_=x_ext[b, :, c0:c1, :],
                    )
                x_mm = x_sb[:]

                # ---- pass 1: z[h, g, v] ----
                y_sb = ypool.tile([128, G, 256], dt_mm, tag="y")
                for g in range(G):
                    zp = zpool.tile([128, 256], f32, tag="z")
                    for cc in range(3):
                        ch = 3 * g + cc
                        if b == 0 and ch < C0:
                            lhs = amat_mm[:, 768 + ch * H: 768 + (ch + 1) * H]
                        else:
                            lhs = x_mm[:, ch, :]
                        nc.tensor.matmul(
                            out=zp[:],
                            lhsT=lhs,
                            rhs=amat_mm[:, cc * 256:(cc + 1) * 256],
                            start=(cc == 0),
                            stop=(cc == 2),
                        )
                    if g % 2 == 0:
                        nc.vector.tensor_copy(y_sb[:, g, :], zp[:])
                    else:
                        nc.scalar.copy(y_sb[:, g, :], zp[:])

                # g -> (G2, gg) view for pass-2 rhs slices
                y_mm = y_sb[:].rearrange(
                    "p (gtwo gg) v -> p gg gtwo v", gg=3)

                # ---- pass 2 + store (raw vblk order; host unscrambles) ----
                # DRAM out layout is [b, r, vh, vblk, c, wv]; copies and DMA
                # descriptors stay fully contiguous, host does the interleave.
                for r in range(2):  # output-row phase: h' = 2*vh + r
                    stage = spool.tile([128, 2 * W * G2], dt_mm, tag="stage")
                    out_view = o_ext[:].rearrange(
                        "b (two vh) w c -> (b two) vh (w c)", two=2)
                    for vblk in range(8):
                        op = opool.tile([128, G2, 32], f32, tag="o2")
                        for gg in range(3):
                            nc.tensor.matmul(
                                out=op[:],
                                lhsT=amat_mm[:, gg * 256 + r * 128: gg * 256 + r * 128 + 128],
                                rhs=y_mm[:, gg, :, vblk * 32:(vblk + 1) * 32],
                                start=(gg == 0),
                                stop=(gg == 2),
                            )
                        dst = stage[:, vblk * 512:(vblk + 1) * 512]
                        if b == B_PER - 1 and r == 1 and vblk == 7:
                            # final copy split across both engines: halves the
                            # last serial PSUM->SBUF latency before the store
                            nc.vector.tensor_copy(
                                stage[:, 7 * 512:7 * 512 + 256], op[:, 0:8, :])
                            nc.scalar.copy(
                                stage[:, 7 * 512 + 256:8 * 512], op[:, 8:16, :])
                        elif vblk % 2 == 0:
                            nc.vector.tensor_copy(dst, op[:])
                        else:
                            nc.scalar.copy(dst, op[:])
                        # batch-0 stores ride the gpsimd SWDGE ring; batch-1
                        # stores use the sync HWDGE ring, which is idle once
                        # the x loads finish -- splits store drain across rings
                        seng = nc.gpsimd if b == 0 else nc.sync
                        if b == B_PER - 1 and r == 1 and vblk >= 4:
                            # final drain: three small sync-ring stores fire as
                            # their chunks complete, so the wire is nearly
                            # drained when the last copy lands
                            if vblk == 5:
                                nc.sync.dma_start(
                                    out=out_view[2 * b + r, :, 4 * 512:6 * 512],
                                    in_=stage[:, 4 * 512:6 * 512])
                            elif vblk == 6:
                                nc.sync.dma_start(
                                    out=out_view[2 * b + r, :, 6 * 512:7 * 512],
                                    in_=stage[:, 6 * 512:7 * 512])
                            elif vblk == 7:
                                nc.sync.dma_start(
                                    out=out_view[2 * b + r, :, 7 * 512:8 * 512],
                                    in_=stage[:, 7 * 512:8 * 512])
                        elif vblk in (3, 7):
                            hw_half = vblk // 4
                            seng.dma_start(
                                out=out_view[2 * b + r, :, hw_half * 2048:(hw_half + 1) * 2048],
                                in_=stage[:, hw_half * 2048:(hw_half + 1) * 2048])

    nc.compile()
    _CACHE["nc"] = nc
    return nc


def _prep_inputs(x: np.ndarray):
    """Host-side input prep: cast to bf16, transpose to [b, w, c, h], and
    build per-core input maps."""
    import ml_dtypes
    x = np.ascontiguousarray(x.transpose(0, 2, 3, 1).astype(ml_dtypes.bfloat16))
    amat = _build_A().transpose(1, 0, 2).reshape(128, 3 * 256).astype(ml_dtypes.bfloat16)
    maps = []
    for i in range(N_CORES):
        xc = x[i * B_PER:(i + 1) * B_PER]
        fused = np.ascontiguousarray(np.concatenate(
            [amat, xc[0, :, 0:C0, :].reshape(128, C0 * H)], axis=1))
        maps.append({"x": xc, "amat": fused})
    return maps


def kernel(x: np.ndarray) -> np.ndarray:
    from concourse.bass_utils import run_bass_kernel_spmd

    assert x.shape == (B_FULL, H, W, C), x.shape
    nc = _get_nc()
    in_maps = _prep_inputs(x)
    res = run_bass_kernel_spmd(nc, in_maps, list(range(N_CORES)))
    out = np.concatenate([res.results[i]["out"] for i in range(N_CORES)], axis=0)
    # device layout [b, r, vh, par, wb, c, wv] -> [b, 2*vh+r, wb*64+wv*2+par, c]
    out = out.reshape(B_FULL, 2, 128, 2, 4, 16, 32)
    out = out.transpose(0, 2, 1, 4, 6, 3, 5).reshape(B_FULL, 2 * H, 2 * W, G2)
    return out.astype(np.float32)
